# revision 3
# baseline (speedup 1.0000x reference)
"""CTLSTM cell fused kernel for 8 Trainium2 NeuronCores.

Strategy (data-parallel over batch):
  - B=16384 rows sharded 2048/core; weights replicated.
  - Host stages transposed operands so the K contraction dim lands on SBUF
    partitions: xh = [x;ht].T -> [1024, 2048/core], w2 = [Wx;Wh].T ->
    [1024, 3584].  One fused GEMM (K=1024, float32r -> 1 col/cycle on PE)
    produces all 7 gate pre-activations per 128-row subtile in PSUM; bias
    is added on DVE from a broadcast bias tile; gates activate in place on
    ACT.
  - softplus(wd) has no ACT table set; computed as -ln(sigmoid(-wd)).
    sigmoid(-wd) is produced in the main pass (same sigmoid/tanh table
    set) and parked in the decay output in DRAM; a second pass reloads it,
    applies Ln (one activation-table switch, forced to the end via
    explicit deps) and a negate, and stores the final decay_rate.
"""

import numpy as np

import concourse.bacc as bacc
import concourse.bass as bass
import concourse.mybir as mybir
import concourse.tile as tile
from concourse.tile_rust import add_dep_helper
from concourse.bass_utils import run_bass_kernel_spmd

NCORES = 8
B = 16384
I = 512
H = 512
NG = 7
G = NG * H          # 3584
K2 = I + H          # 1024
P = 128
BS = B // NCORES    # 2048 rows per core
NT = BS // P        # 16 subtiles of 128 rows
SUP = 4             # subtiles per supertile (DMA granularity)
NSUP = NT // SUP

F32R = mybir.dt.float32r
F32 = mybir.dt.float32
AF = mybir.ActivationFunctionType

TRACE = False
LAST_RESULTS = None

_nc_cache = None


def _build():
    nc = bacc.Bacc("TRN2", target_bir_lowering=False, debug=False)

    xh = nc.dram_tensor("xh", [K2, BS], F32R, kind="ExternalInput")
    w2 = nc.dram_tensor("w2", [K2, G], F32R, kind="ExternalInput")
    ct = nc.dram_tensor("ct", [BS, H], F32, kind="ExternalInput")
    bb_d = nc.dram_tensor("bb", [P, G], F32, kind="ExternalInput")

    h_d = nc.dram_tensor("h", [BS, H], F32, kind="ExternalOutput")
    c_d = nc.dram_tensor("c", [BS, H], F32, kind="ExternalOutput")
    cb_d = nc.dram_tensor("cb", [BS, H], F32, kind="ExternalOutput")
    o_d = nc.dram_tensor("o", [BS, H], F32, kind="ExternalOutput")
    dr_d = nc.dram_tensor("dr", [BS, H], F32, kind="ExternalOutput")

    last_sn = None  # final sigmoid(-wd) ACT instruction, gates phase 2

    with tile.TileContext(nc) as tc:
        with (
            tc.tile_pool(name="wp", bufs=1) as wp,
            tc.tile_pool(name="cp", bufs=1) as cp,
            tc.tile_pool(name="xp", bufs=2) as xp,
            tc.tile_pool(name="ctp", bufs=4) as ctp,
            tc.tile_pool(name="gp", bufs=10) as gp,
            tc.tile_pool(name="ph2", bufs=2) as ph2,
            tc.tile_pool(name="pp", bufs=6, space=bass.MemorySpace.PSUM) as pp,
        ):
            # resident weights: 8 K-chunks of [128, 3584]
            w_sb = []
            for k in range(8):
                wt = wp.tile([P, G], F32R, tag=f"w{k}")
                nc.sync.dma_start(wt[:], w2[k * P:(k + 1) * P, :])
                w_sb.append(wt)
            # broadcast bias [128, 3584] (bx+bh, host-staged broadcast)
            bb = cp.tile([P, G], F32, tag="bb")
            nc.sync.dma_start(bb[:], bb_d[:])

            for s in range(NSUP):
                xhs = []
                for k in range(8):
                    t_ = xp.tile([P, SUP * P], F32R, tag=f"xh{k}")
                    nc.sync.dma_start(
                        t_[:], xh[k * P:(k + 1) * P, s * SUP * P:(s + 1) * SUP * P]
                    )
                    xhs.append(t_)

                for j in range(SUP):
                    t = s * SUP + j
                    bsl = slice(j * P, (j + 1) * P)
                    rows = slice(t * P, (t + 1) * P)

                    ctj = ctp.tile([P, H], F32, tag="ct")
                    nc.sync.dma_start(ctj[:], ct[rows, :])

                    g_tiles = []
                    for g in range(NG):
                        acc = pp.tile([P, H], F32, tag="acc")
                        for k in range(8):
                            nc.tensor.matmul(
                                acc[:],
                                xhs[k][:, bsl],
                                w_sb[k][:, g * H:(g + 1) * H],
                                start=(k == 0),
                                stop=(k == 7),
                            )
                        gt = gp.tile([P, H], F32, tag="g")
                        nc.vector.tensor_add(gt[:], acc[:], bb[:, g * H:(g + 1) * H])
                        g_tiles.append(gt)

                    gi, gf, gz, go, gd, gib, gfb = g_tiles
                    nc.scalar.activation(gi[:], gi[:], AF.Sigmoid)
                    nc.scalar.activation(gf[:], gf[:], AF.Sigmoid)
                    nc.scalar.activation(gz[:], gz[:], AF.Tanh)
                    nc.scalar.activation(go[:], go[:], AF.Sigmoid)
                    nc.scalar.activation(gib[:], gib[:], AF.Sigmoid)
                    nc.scalar.activation(gfb[:], gfb[:], AF.Sigmoid)
                    last_sn = nc.scalar.activation(gd[:], gd[:], AF.Sigmoid,
                                                   scale=-1.0)
                    # park sigmoid(-wd) in the decay output; phase 2 rewrites it
                    nc.sync.dma_start(dr_d[rows, :], gd[:])

                    nc.sync.dma_start(o_d[rows, :], go[:])

                    nc.vector.tensor_mul(gf[:], gf[:], ctj[:])     # f*ct
                    nc.vector.tensor_mul(gi[:], gi[:], gz[:])      # i*z
                    nc.vector.tensor_add(gf[:], gf[:], gi[:])      # c
                    nc.sync.dma_start(c_d[rows, :], gf[:])
                    nc.vector.tensor_mul(gib[:], gib[:], gz[:])    # ib*z
                    nc.scalar.activation(gz[:], gf[:], AF.Tanh)    # tanh(c)
                    nc.vector.tensor_mul(gfb[:], gfb[:], ctj[:])   # fb*ct
                    nc.vector.tensor_add(gfb[:], gfb[:], gib[:])   # cbar
                    nc.sync.dma_start(cb_d[rows, :], gfb[:])
                    nc.vector.tensor_mul(gz[:], go[:], gz[:])      # h = o*tanh(c)
                    nc.sync.dma_start(h_d[rows, :], gz[:])

            # phase 2: decay_rate = softplus(wd) = -ln(sigmoid(-wd))
            dr_r = dr_d.rearrange("(n t p) c -> n p t c", t=SUP, p=P)
            for chn in range(NSUP):
                sn = ph2.tile([P, SUP, H], F32, tag="sn")
                ld = nc.sync.dma_start(sn[:], dr_r[chn])
                # keep the reload (and thus Ln) after every main-pass ACT so
                # the activation table switches exactly once
                add_dep_helper(ld.ins, last_sn.ins, reason="phase2 after phase1")
                nc.scalar.activation(sn[:], sn[:], AF.Ln)
                nc.vector.tensor_scalar_mul(sn[:], sn[:], -1.0)
                nc.sync.dma_start(dr_r[chn], sn[:])

    nc.compile()
    return nc


def kernel(x, ht, ct, Wx, bx, Wh, bh):
    global _nc_cache, LAST_RESULTS
    if _nc_cache is None:
        _nc_cache = _build()
    nc = _nc_cache

    x = np.ascontiguousarray(x, dtype=np.float32)
    ht = np.ascontiguousarray(ht, dtype=np.float32)
    ct = np.ascontiguousarray(ct, dtype=np.float32)

    # host staging: transposed/concatenated operands, broadcast bias
    xh_full = np.empty((K2, B), dtype=np.float32)
    xh_full[:I, :] = x.T
    xh_full[I:, :] = ht.T
    w2 = np.empty((K2, G), dtype=np.float32)
    w2[:I, :] = np.asarray(Wx, dtype=np.float32).T
    w2[I:, :] = np.asarray(Wh, dtype=np.float32).T
    bsum = (np.asarray(bx, dtype=np.float32) + np.asarray(bh, dtype=np.float32))
    bb = np.ascontiguousarray(np.broadcast_to(bsum[None, :], (P, G)))

    in_maps = []
    for cidx in range(NCORES):
        sl = slice(cidx * BS, (cidx + 1) * BS)
        in_maps.append({
            "xh": np.ascontiguousarray(xh_full[:, sl]),
            "w2": w2,
            "ct": ct[sl],
            "bb": bb,
        })

    res = run_bass_kernel_spmd(nc, in_maps, core_ids=list(range(NCORES)),
                               trace=TRACE)
    LAST_RESULTS = res

    outs = {}
    for name in ("h", "c", "cb", "o", "dr"):
        outs[name] = np.concatenate(
            [res.results[cidx][name] for cidx in range(NCORES)], axis=0
        )
    return outs["h"], outs["c"], outs["cb"], outs["o"], outs["dr"]
